# revision 29
# baseline (speedup 1.0000x reference)
"""Bahdanau additive attention on 8 TRN2 NeuronCores, data-parallel over batch.

reference:
    h1 = enc @ W1 + b1              [B,S,U]
    h2 = hid @ W2 + b2              [B,1,U]
    score = tanh(h1+h2) @ V + bv    [B,S,1]   (bv dropped: softmax-invariant)
    w = softmax(score, axis=S)
    ctx = sum_s w * enc             [B,D]

Sharding: data-parallel over batch, 4 batches per core, weights replicated,
no collectives.

Structure: t-blocks processed in PAIRS. Each m-step computes h1 for
both t's of the pair into one [P, 2*NT] 2-bank PSUM tile, so tanh and the
V-FMA run as single double-width ops: ACT/DVE ops on TRN2 pay a ~380ns
init bubble each (errata), and the v5 profile showed the pipeline paced by
DVE at 9us/t-block with the PE stalled on cross-engine semaphores 80% of
its "busy" time. Halving the op count on ACT/DVE while keeping PE work
identical puts the PE back in charge. Measured 250us wall vs the 321us
bf16 starting point; steady-state pairs run at ~98% of the PE's
instruction-stream floor (DR matmul = 215ns per 256-deep contraction =
the 2-byte/cycle/partition moving-operand bound; LDWEIGHTS fully hidden).
Head (~30us) is enc-DMA pacing x the slow (~25us) DVFS ramp to the full
p-state; tail (~20us) is mostly fixed NEFF teardown. Warm-up/keep-warm
dummy matmuls (scratch operands, ph1-ring PSUM) shave a little of both.

Details:
  - whole per-core enc cast-DMA'd f32->bf16 (SWDGE ring paces ~1.65us/tile
    steady, ~0.7us burst) into SBUF up front; tiles stay resident for the
    ctx pass (128KB/partition). Identity/ones come from DRAM so gpsimd's
    queue isn't blocked behind iota/affine_select (the NEFF preamble
    already eats ~7.8us before gpsimd's first DMA issue).
  - encT [d, s] built by PE matmuls against a bf16 identity (~46ns/op);
    PSUM->SBUF copies cast to fp8e4 (DVE/ACT alternating). The next pair's
    transposes are spread 2-per-m-step across the current pair's m-loop so
    they never head-block h1 in the in-order PE queue.
  - h1T = W1.T @ encT in fp8 DoubleRow perf mode (256-deep contraction per
    matmul; ~2x bf16 on silicon, up to ~107ns/matmul when nothing stalls).
    Precision: plain fp8 h1 is 2.45e-2 > 2e-2 gate, so the u-axis is
    permuted by |V| descending (host side, consistently for W1/bias/V) and
    a W1lo = e5m2(W1 - W1hi) correction pass runs for the top 256 u only
    (first 2 of 8 m-chunks, 72% of sum V^2). Host-emulated end-to-end
    rel_err 1.787e-2; measured 1.771e-2 (emulation tracks hardware to
    ~0.02e-2, and inputs/NEFF are deterministic, so the margin holds).
    enc stays single-level e4m3.
  - m-loop runs hi-only chunks (4..7) first so the first matmuls of the
    run need only w1hi; the w1lo DMAs land meanwhile.
  - ScalarE tanh (double-width, per-partition bias; h2+b1+b2 on host).
  - vacc = sum_m V_m*tanh_m as double-width DVE FMA; score columns via
    ones-matmuls on vacc chunks into a per-pair [P, 8] PSUM tile carved
    from the ps_h1 ring (PSUM is exactly 8 banks: 2x2 ph1 + 2 transpose
    + 2 ctx).
  - incremental softmax per pair: exp (unnormalized, accum_out rowsum
    partial) -> ctx partial matmuls accumulate into PSUM immediately; the
    global 1/sum chain overlaps the last ctx matmuls. No serial tail.
  - ctx = esc.T @ enc_native from the cached bf16 tiles, scaled by 1/sum.
"""
import sys
import numpy as np
from contextlib import ExitStack

if "/opt/trn_rl_repo" not in sys.path:
    sys.path.insert(0, "/opt/trn_rl_repo")

import ml_dtypes
from concourse import bacc, mybir, tile
from concourse.bass_utils import run_bass_kernel_spmd

F32 = mybir.dt.float32
BF16 = mybir.dt.bfloat16
FP8E4 = mybir.dt.float8e4
FP8E5 = mybir.dt.float8e5
BF16NP = ml_dtypes.bfloat16
E4NP = ml_dtypes.float8_e4m3
E5NP = ml_dtypes.float8_e5m2
DR = mybir.MatmulPerfMode.DoubleRow

B, S, D, U = 32, 2048, 1024, 1024
NCORES = 8
BL = B // NCORES          # 4 batches per core
P = 128
KD = D // P               # 8 d-chunks
KU = U // P               # 8 u-chunks
NT = 512                  # matmul free-dim tile
ST = S // NT              # 4 s-tiles per batch
NH = ST // 2              # 2 t-pairs per batch
SB = S // P               # 16 s-blocks of 128
LOC = 2                   # m-chunks with the W1lo correction (top-256 u)
LOW = LOC * P

_NC_CACHE = None
LAST_RESULT = None        # test.py reads exec_time_ns off this
TRACE_DIR = None          # when set (and BASS_TRACE=1), ntff profile lands here


def _build():
    nc = bacc.Bacc("TRN2", target_bir_lowering=False)

    enc_in = nc.dram_tensor("enc", [BL, S, D], F32, kind="ExternalInput")
    ident_in = nc.dram_tensor("ident", [P, P], BF16, kind="ExternalInput")
    ones_in = nc.dram_tensor("ones", [P, 1], BF16, kind="ExternalInput")
    w1hi_in = nc.dram_tensor("w1hi", [P, KD, U], FP8E4, kind="ExternalInput")
    w1lo_in = nc.dram_tensor("w1lo", [P, KD, LOW], FP8E5, kind="ExternalInput")
    bias_in = nc.dram_tensor("biasT", [P, KU * BL], F32, kind="ExternalInput")
    vT_in = nc.dram_tensor("vT", [P, KU], F32, kind="ExternalInput")
    out_ext = nc.dram_tensor("out", [BL, D], F32, kind="ExternalOutput")

    with tile.TileContext(nc) as tc, ExitStack() as ctx:
        const = ctx.enter_context(tc.tile_pool(name="const", bufs=1))
        nat_pool = ctx.enter_context(tc.tile_pool(name="nat", bufs=BL * SB))
        encT_pool = ctx.enter_context(tc.tile_pool(name="encT", bufs=4))
        tanh_pool = ctx.enter_context(tc.tile_pool(name="tanh", bufs=3))
        vacc_pool = ctx.enter_context(tc.tile_pool(name="vacc", bufs=2))
        small = ctx.enter_context(tc.tile_pool(name="small", bufs=4))
        out_pool = ctx.enter_context(tc.tile_pool(name="outp", bufs=2))

        # PSUM: exactly 8 banks. ph1 tiles are 2-bank [P, 2, NT]; the
        # per-pair score tile and per-batch sum tile ride the same ring.
        ps_h1 = ctx.enter_context(tc.tile_pool(name="ps_h1", bufs=2, space="PSUM"))
        ps_tr = ctx.enter_context(tc.tile_pool(name="ps_tr", bufs=2, space="PSUM"))
        ps_ctx = ctx.enter_context(tc.tile_pool(name="ps_ctx", bufs=1, space="PSUM"))

        # ---- PE clock warm-up: the PE DVFS sits at the mid p-state (2x
        # slower) until it sees a few us of continuous work, and the whole
        # DMA-gated ramp otherwise runs at half clock. Scratch matmuls with
        # no DMA dependency keep it busy from t~=5us until enc arrives.
        scratch = const.tile([P, P], BF16)
        nc.any.memset(scratch[:], 1.0)

        def emit_warm(n):
            # rides the ph1 ring (tag ph1) so it never contends with the
            # transpose/score tiles in ps_tr.
            wps = ps_h1.tile([P, NT], F32, tag="ph1", name=f"warm{emit_warm.i}")
            emit_warm.i += 1
            for _ in range(n):
                nc.tensor.matmul(wps[:, :P], scratch[:], scratch[:],
                                 start=True, stop=True)
        emit_warm.i = 0

        # ---- whole-core enc prefetch first: the SWDGE ring paces tile
        # arrival, so anything ahead of these issues delays batch 0.
        nat_tiles = {}
        for b in range(BL):
            for st in range(SB):
                nt_t = nat_pool.tile([P, D], BF16, name=f"nat_{b}_{st}",
                                     tag="nat")
                nc.gpsimd.dma_start(nt_t[:], enc_in[b, st * P:(st + 1) * P, :])
                nat_tiles[(b, st)] = nt_t

        # ---- constants, ordered by first-use time on the scalar queue ----
        ident = const.tile([P, P], BF16)
        nc.scalar.dma_start(ident[:], ident_in[:])
        ones128 = const.tile([P, 1], BF16)
        nc.scalar.dma_start(ones128[:], ones_in[:])
        v32_sb = const.tile([P, KU], F32)
        nc.scalar.dma_start(v32_sb[:], vT_in[:])
        bias_sb = const.tile([P, KU * BL], F32)   # bias[u(m,p), m*BL+b]
        nc.scalar.dma_start(bias_sb[:], bias_in[:])
        w1hi_sb = const.tile([P, KD, U], FP8E4)
        for k in range(KD):
            eng = nc.sync if k % 2 == 0 else nc.scalar
            eng.dma_start(w1hi_sb[:, k, :], w1hi_in[:, k, :])
        w1lo_sb = const.tile([P, KD, LOW], FP8E5)
        for k in range(KD):
            eng = nc.sync if k % 2 == 0 else nc.scalar
            eng.dma_start(w1lo_sb[:, k, :], w1lo_in[:, k, :])

        def emit_transpose_k(b, t, encT, k):
            """encT[:, k, j*128:(j+1)*128] = nat[b][t*4+j][:, k*128:(k+1)*128].T

            Regular matmuls against the identity (out = natchunk.T @ I)
            pipeline at ~46ns/op; the PSUM->SBUF copy casts to fp8e4.
            """
            pt = ps_tr.tile([P, NT], F32, tag="pt")
            for j in range(NT // P):
                nc.tensor.matmul(
                    pt[:, j * P:(j + 1) * P],
                    nat_tiles[(b, t * (NT // P) + j)][:, k * P:(k + 1) * P],
                    ident[:], start=True, stop=True)
            if k % 2 == 0:
                nc.vector.tensor_copy(encT[:, k, :], pt[:])
            else:
                nc.scalar.activation(encT[:, k, :], pt[:],
                                     mybir.ActivationFunctionType.Copy)

        # hi-only m-chunks first: their matmuls need only w1hi.
        M_ORDER = list(range(KU // 2, KU)) + list(range(KU // 2))

        emit_warm(60)

        # encT tile for t=0 of batch 0 only, before the main loop; t=1's
        # transposes are spread through t=0's ramp m-loop so h1 starts
        # after just 4 enc tiles have landed.
        encTs = {}
        encTs[0] = encT_pool.tile([P, KD, NT], FP8E4, name="encT_p0", tag="encT")
        for k in range(KD):
            emit_transpose_k(0, 0, encTs[0], k)

        for b in range(BL):
            pc = [ps_ctx.tile([1, NT], F32, name=f"pc{h}", tag=f"pc{h}")
                  for h in range(D // NT)]
            esc = small.tile([P, SB], BF16, name=f"esc{b}", tag="esc")
            rowsums = small.tile([P, NH], F32, name=f"rsum{b}", tag="rsum")
            for half in range(NH):
                t0, t1 = 2 * half, 2 * half + 1
                ramp = (b == 0 and half == 0)
                encT0 = encTs.pop(t0)
                if ramp:
                    encT1 = encT_pool.tile([P, KD, NT], FP8E4, name="encT1r", tag="encT")
                else:
                    encT1 = encTs.pop(t1)
                # next pair's transposes spread over this pair's m-loop
                nxt = None
                if not (b == BL - 1 and half == NH - 1):
                    nb, nt0 = (b, 2) if half == 0 else (b + 1, 0)
                    nxt = (nb, nt0)
                    nx0 = encT_pool.tile([P, KD, NT], FP8E4, name=f"nx0_{b}_{half}", tag="encT")
                    nx1 = encT_pool.tile([P, KD, NT], FP8E4, name=f"nx1_{b}_{half}", tag="encT")
                vacc = vacc_pool.tile([P, 2, NT], BF16)

                def h1_mms(ph1_out, encTx, m, has_lo):
                    for kk in range(KD // 2):
                        nc.tensor.matmul(
                            ph1_out,
                            w1hi_sb[:, 2 * kk:2 * kk + 2, m * P:(m + 1) * P],
                            encTx[:, 2 * kk:2 * kk + 2, :],
                            start=(kk == 0),
                            stop=(not has_lo and kk == KD // 2 - 1),
                            perf_mode=DR)
                    if has_lo:
                        for kk in range(KD // 2):
                            nc.tensor.matmul(
                                ph1_out,
                                w1lo_sb[:, 2 * kk:2 * kk + 2, m * P:(m + 1) * P],
                                encTx[:, 2 * kk:2 * kk + 2, :],
                                start=False, stop=(kk == KD // 2 - 1),
                                perf_mode=DR)

                if ramp:
                    # batch 0's first pair runs t0 then t1 sequentially so
                    # h1 starts once 4 enc tiles have landed; t1's (then
                    # the next pair's) transposes spread over the m-steps.
                    for ti in (0, 1):
                        encTx = encT0 if ti == 0 else encT1
                        for mi, m in enumerate(M_ORDER):
                            ph1 = ps_h1.tile([P, NT], F32, tag="ph1")
                            h1_mms(ph1[:], encTx, m, m < LOC)
                            tanh_t = tanh_pool.tile([P, NT], BF16)
                            nc.scalar.activation(
                                tanh_t[:], ph1[:],
                                mybir.ActivationFunctionType.Tanh,
                                bias=bias_sb[:, m * BL + b:m * BL + b + 1],
                                scale=1.0)
                            if mi == 0:
                                nc.vector.tensor_scalar_mul(
                                    vacc[:, ti, :], tanh_t[:],
                                    v32_sb[:, m:m + 1])
                            else:
                                nc.vector.scalar_tensor_tensor(
                                    vacc[:, ti, :], tanh_t[:],
                                    v32_sb[:, m:m + 1], vacc[:, ti, :],
                                    mybir.AluOpType.mult, mybir.AluOpType.add)
                            if ti == 0:
                                emit_transpose_k(0, 1, encT1, mi)
                            else:
                                for k in (2 * (mi % 4), 2 * (mi % 4) + 1):
                                    emit_transpose_k(0, 2 + mi // 4,
                                                     nx0 if mi < 4 else nx1, k)
                else:
                    for mi, m in enumerate(M_ORDER):
                        ph1 = ps_h1.tile([P, 2, NT], F32, tag="ph1")
                        has_lo = m < LOC
                        for ti, encTx in ((0, encT0), (1, encT1)):
                            h1_mms(ph1[:, ti, :], encTx, m, has_lo)
                        tanh_t = tanh_pool.tile([P, 2, NT], BF16)
                        nc.scalar.activation(
                            tanh_t[:], ph1[:],
                            mybir.ActivationFunctionType.Tanh,
                            bias=bias_sb[:, m * BL + b:m * BL + b + 1],
                            scale=1.0)
                        if mi == 0:
                            nc.vector.tensor_scalar_mul(
                                vacc[:], tanh_t[:], v32_sb[:, m:m + 1])
                        else:
                            nc.vector.scalar_tensor_tensor(
                                vacc[:], tanh_t[:], v32_sb[:, m:m + 1], vacc[:],
                                mybir.AluOpType.mult, mybir.AluOpType.add)
                        if nxt is not None:
                            nb, nt0 = nxt
                            for k in (2 * (mi % 4), 2 * (mi % 4) + 1):
                                emit_transpose_k(nb, nt0 + mi // 4,
                                                 nx0 if mi < 4 else nx1, k)
                # score columns for this pair: contract vacc chunks with ones
                if b == BL - 1 and half == NH - 1:
                    emit_warm(16)
                psum_sT = ps_tr.tile([P, 2 * (NT // P)], F32, tag="pt")
                for jj in range(2 * (NT // P)):
                    nc.tensor.matmul(
                        psum_sT[:, jj:jj + 1],
                        vacc[:, jj // (NT // P), (jj % (NT // P)) * P:
                             (jj % (NT // P)) * P + P],
                        ones128[:, :1], start=True, stop=True)
                nc.scalar.activation(
                    esc[:, half * 2 * (NT // P):(half + 1) * 2 * (NT // P)],
                    psum_sT[:],
                    mybir.ActivationFunctionType.Exp,
                    accum_out=rowsums[:, half:half + 1])
                if b == BL - 1 and half == NH - 1:
                    emit_warm(20)
                if half == NH - 1:
                    # 1/sum chain before this pair's ctx matmuls: the
                    # reciprocal overlaps them on the PE queue.
                    rowsum = small.tile([P, 1], F32, name=f"rowsum{b}",
                                        tag="rowsum")
                    nc.vector.tensor_tensor(
                        rowsum[:], rowsums[:, 0:1], rowsums[:, 1:2],
                        mybir.AluOpType.add)
                    rs_bf = small.tile([P, 1], BF16, name=f"rs_bf{b}",
                                       tag="rs_bf")
                    nc.vector.tensor_copy(rs_bf[:], rowsum[:])
                    psum_s1 = ps_tr.tile([1, 1], F32, tag="pt")
                    nc.tensor.matmul(psum_s1[:], rs_bf[:, :], ones128[:, :1],
                                     start=True, stop=True)
                    sum_sb = small.tile([1, 1], F32, name=f"sum_sb{b}",
                                        tag="sum_sb")
                    nc.vector.tensor_copy(sum_sb[:], psum_s1[:])
                    rinv = small.tile([1, 1], F32, name=f"rinv{b}", tag="rinv")
                    nc.vector.reciprocal(rinv[:], sum_sb[:])
                for jj in range(2 * (NT // P)):
                    j = half * 2 * (NT // P) + jj
                    for h in range(D // NT):
                        nc.tensor.matmul(
                            pc[h][:], esc[:, j:j + 1],
                            nat_tiles[(b, j)][:, h * NT:(h + 1) * NT],
                            start=(j == 0), stop=(j == SB - 1))
                if nxt is not None:
                    nb, nt0 = nxt
                    encTs[nt0] = nx0
                    encTs[nt0 + 1] = nx1
            out_t = out_pool.tile([1, D], F32, name=f"out_t{b}", tag="out_t")
            for h in range(D // NT):
                nc.vector.tensor_scalar_mul(
                    out_t[:1, h * NT:(h + 1) * NT], pc[h][:], rinv[:1, :1])
            nc.gpsimd.dma_start(out_ext[b:b + 1, :], out_t[:1, :])

    nc.compile()
    return nc


def _get_nc():
    global _NC_CACHE
    if _NC_CACHE is None:
        _NC_CACHE = _build()
    return _NC_CACHE


def kernel(**inputs):
    global LAST_RESULT
    enc = np.asarray(inputs["enc"], dtype=np.float32)
    hid = np.asarray(inputs["hid"], dtype=np.float32)
    W1 = np.asarray(inputs["W1"], dtype=np.float32)
    b1 = np.asarray(inputs["b1"], dtype=np.float32)
    W2 = np.asarray(inputs["W2"], dtype=np.float32)
    b2 = np.asarray(inputs["b2"], dtype=np.float32)
    V = np.asarray(inputs["V"], dtype=np.float32)
    # bv shifts all scores of a batch equally -> softmax unchanged; unused.

    # host-side layout prep (pure reshapes/casts of tiny tensors).
    # u-axis permuted by |V| descending so the fp8 lo-correction pass can
    # cover only the top-512 u (they carry ~93% of sum V^2).
    perm = np.argsort(-np.abs(V[:, 0]))
    W1p = np.ascontiguousarray(W1[:, perm])
    Vp = V[perm, 0]
    w1r = np.ascontiguousarray(
        W1p.reshape(KD, P, U).transpose(1, 0, 2))            # [P, KD, U] f32
    w1hi = w1r.astype(E4NP)
    w1lo = (w1r[:, :, :LOW]
            - w1hi[:, :, :LOW].astype(np.float32)).astype(E5NP)
    vT = np.ascontiguousarray(Vp.reshape(KU, P).T)
    # h2+biases on host: 67 MFLOP, 0.05% of the device work
    bias_full = (hid @ W2 + b2 + b1).astype(np.float32)[:, perm]  # [B, U]

    ident = np.eye(P, dtype=BF16NP)
    ones = np.ones((P, 1), dtype=BF16NP)

    nc = _get_nc()
    in_maps = []
    for i in range(NCORES):
        bs = bias_full[i * BL:(i + 1) * BL]                  # [BL, U]
        biasT = np.ascontiguousarray(
            bs.reshape(BL, KU, P).transpose(2, 1, 0).reshape(P, KU * BL))
        in_maps.append({
            "enc": np.ascontiguousarray(enc[i * BL:(i + 1) * BL]),
            "ident": ident, "ones": ones,
            "w1hi": w1hi, "w1lo": w1lo, "biasT": biasT, "vT": vT,
        })
    kwargs = {}
    if TRACE_DIR is not None:
        kwargs["tmpdir"] = TRACE_DIR
    res = run_bass_kernel_spmd(nc, in_maps, list(range(NCORES)), **kwargs)
    LAST_RESULT = res
    out = np.concatenate([res.results[i]["out"] for i in range(NCORES)], axis=0)
    return out.astype(np.float32)
